# revision 11
# baseline (speedup 1.0000x reference)
"""Trainium2 Bass kernel for nn_Encoder_66331474919809.

6-layer encoder, each layer: per-head attention scores -> per-head sparse
autoencoder (relu/sigmoid) -> softmax -> attn, out-proj, LN, FFN (double
relu), LN.  B=4, T=1024, E=512, H=8, HD=64, T4=256, F=2048.

Sharding: 8 cores = (batch b, query-half q).  Each core computes all 8
heads for its 512 query rows; keys/values span all 1024 positions (k/v
projections are computed redundantly on both cores of a pair).  One
pairwise AllGather per layer exchanges the layer output halves.

Key implementation choices:
- scores/SAE activations kept transposed as [key|latent (partitions),
  query (free)] so every additive bias (bq,bk,be,bd,b1) is a
  per-partition scalar consumed for free by ScalarE/VectorE ops;
- softmax denominator produced by augmenting V with a ones column
  inside the attn x V matmul (psum row 64 = sum of exp);
- sigmoid computed as 0.5 + 0.5*tanh(z/2) so the whole SAE nonlinearity
  chain (tanh+exp+relu+copy) lives in one ScalarE activation-table set
  (avoids ~1.3us table reloads per switch);
- weights host-pre-tiled into a handful of big contiguous DRAM arrays
  (one DMA per layer for qkv / ffn / wo / biases, one per head for SAE);
- embedding gather, q-scaling, and all weight layout transforms on host.
"""

import sys

for _p in ("/opt/trn_rl_repo", "/root/.axon_site/_ro/trn_rl_repo"):
    if _p not in sys.path:
        sys.path.insert(0, _p)

import numpy as np
import ml_dtypes

import concourse.bass as bass
import concourse.tile as tile
from concourse import bacc, mybir
from concourse.bass_utils import run_bass_kernel_spmd
from concourse.masks import make_identity

L, V, E, D, H = 6, 32000, 512, 512, 8
HD = D // H
T = 1024
T4 = T // 4
F = 2048
B = 4
Tq = T // 2          # query rows per core
NCORES = 8
EPS = 1e-5
LAM = 1e-3
SCALE = float(np.float32(np.sqrt(1.0 / HD)))

F32 = mybir.dt.float32
AF = mybir.ActivationFunctionType
ALU = mybir.AluOpType

_BUILD_CACHE = {}


def _build(n_layers=L, use_bf16=True, affine=False, biases=False):
    """Build + finalize the SPMD Bass program (same NEFF on all 8 cores).

    affine: apply LayerNorm gamma/beta (setup_inputs uses ones/zeros).
    biases: apply projection biases (setup_inputs uses zeros).
    """
    DT = mybir.dt.bfloat16 if use_bf16 else mybir.dt.float32

    nc = bacc.Bacc("TRN2", target_bir_lowering=False, debug=False,
                   enable_asserts=True, num_devices=NCORES)

    # ---- DRAM I/O (host-pre-tiled consolidated layouts) ----
    hT0 = nc.dram_tensor("hT0", [E, T], DT, kind="ExternalInput")
    qhT0 = nc.dram_tensor("qhT0", [E, Tq], DT, kind="ExternalInput")
    h0h = nc.dram_tensor("h0h", [Tq, E], F32, kind="ExternalInput")
    # [128, 6176]: wq lhsT (pair p, chunk c) at p*512+c*128, wk at +2048,
    # wv-aug pair rhs (pair p, chunk c) 130 wide at 4096+p*520+c*130
    qkvw = nc.dram_tensor("qkvw", [n_layers, 128, 6176], DT, kind="ExternalInput")
    # per head [128, 4096]: we lhsT (sc,uc) at sc*256+uc*128,
    # wd lhsT (uc,sc) at 2048+uc*1024+sc*128
    saew = nc.dram_tensor("saew", [n_layers, H, 128, 4096], DT, kind="ExternalInput")
    # [128, 2048]: wo rhs (pair p) at p*512
    wo1 = nc.dram_tensor("wo1", [n_layers, 128, 2048], DT, kind="ExternalInput")
    # [128, 16384]: w1 lhsT (c,fc) at c*2048+fc*128; w2 rhs (fc) at 8192+fc*512
    ffnw = nc.dram_tensor("ffnw", [n_layers, 128, 16384], DT, kind="ExternalInput")
    # [128, 104] f32: bq(p) @0+p, bk(p) @4+p, be(h,uc) @8+h*2+uc,
    # 0.5*bd(h,sc) @24+h*8+sc, b1(fc) @88+fc
    b128 = nc.dram_tensor("b128", [n_layers, 128, 104], F32, kind="ExternalInput")
    # [1, 1544] DT: bva(h) @h*65 (65 wide), bo @520, b2 @1032
    brow = nc.dram_tensor("brow", [n_layers, 1, 1544], DT, kind="ExternalInput")
    # [4, 512] f32: ln1_g, ln1_b, ln2_g, ln2_b (broadcast-loaded)
    lnw = nc.dram_tensor("lnw", [n_layers, 4, E], F32, kind="ExternalInput")
    hout = nc.dram_tensor("hout", [Tq, E], F32, kind="ExternalOutput")

    agin = [nc.dram_tensor(f"agin{l}", [Tq, E], DT) for l in range(n_layers)]
    agout = [nc.dram_tensor(f"agout{l}", [T, E], DT) for l in range(n_layers)]
    groups = [[0, 1], [2, 3], [4, 5], [6, 7]]

    with tile.TileContext(nc) as tc:
        from contextlib import ExitStack
        with ExitStack() as ctx:
            cpool = ctx.enter_context(tc.tile_pool(name="const", bufs=1))
            wpool = ctx.enter_context(tc.tile_pool(name="wts", bufs=1))
            bpool = ctx.enter_context(tc.tile_pool(name="bias", bufs=2))
            hpool = ctx.enter_context(tc.tile_pool(name="hstate", bufs=5))
            apool = ctx.enter_context(tc.tile_pool(name="acts", bufs=2))
            spool = ctx.enter_context(tc.tile_pool(name="small", bufs=2))
            # PSUM budget (8 banks): pacc 3 + plat 2 + pot 2 = 7
            pp = ctx.enter_context(tc.tile_pool(name="ps", bufs=3, space="PSUM"))

            ones_col = cpool.tile([1, 128], DT, tag="ones")
            nc.vector.memset(ones_col[:], 1.0)
            eps_t = cpool.tile([128, 1], F32, tag="eps")
            nc.vector.memset(eps_t[:], EPS)
            half_t = cpool.tile([128, 1], F32, tag="half")
            nc.vector.memset(half_t[:], 0.5)
            ident = cpool.tile([128, 128], F32, tag="ident")
            make_identity(nc, ident[:])

            def ln_apply(x_ap, y_tile, g_bc, b_bc):
                """LayerNorm over the free axis (512) of x_ap -> y_tile."""
                stats = spool.tile([128, 6], F32, tag="lnstat", name="lnstat")
                nc.vector.bn_stats(out=stats[:], in_=x_ap)
                mv = spool.tile([128, 2], F32, tag="lnmv", name="lnmv")
                nc.vector.bn_aggr(out=mv[:], in_=stats[:])
                rs = spool.tile([128, 1], F32, tag="lnrs", name="lnrs")
                nc.scalar.activation(out=rs[:], in_=mv[:, 1:2], func=AF.Sqrt,
                                     bias=eps_t[:])
                nc.vector.reciprocal(out=rs[:], in_=rs[:])
                nc.vector.tensor_scalar(out=y_tile[:], in0=x_ap,
                                        scalar1=mv[:, 0:1], scalar2=rs[:],
                                        op0=ALU.subtract, op1=ALU.mult)
                if affine:
                    nc.vector.tensor_mul(out=y_tile[:], in0=y_tile[:], in1=g_bc)
                    nc.vector.tensor_add(out=y_tile[:], in0=y_tile[:], in1=b_bc)

            def transpose_rows(y_tiles, out_tiles, key):
                """PE-transpose 4x [128,E] f32 row tiles -> 4x [128,512] DT
                column tiles (out[ec][:, tc*128:...] = y[tc][:, ec*128:...].T)."""
                for tc_ in range(4):
                    for ec in range(4):
                        ptt = pp.tile([128, 128], F32, tag="pacc",
                                      name=f"ptt{key}_{tc_}_{ec}")
                        nc.tensor.transpose(
                            ptt[:], y_tiles[tc_][:, ec * 128:(ec + 1) * 128],
                            ident[:])
                        nc.vector.tensor_copy(
                            out=out_tiles[ec][:, tc_ * 128:(tc_ + 1) * 128],
                            in_=ptt[:])

            # persistent state
            hT = []
            for ec in range(4):
                t_ = hpool.tile([128, T], DT, tag="hT", name=f"hT{ec}")
                nc.sync.dma_start(out=t_[:], in_=hT0[ec * 128:(ec + 1) * 128, :])
                hT.append(t_)
            hQ = []
            for ec in range(4):
                t_ = hpool.tile([128, Tq], DT, tag="hQ", name=f"hQ{ec}")
                nc.sync.dma_start(out=t_[:], in_=qhT0[ec * 128:(ec + 1) * 128, :])
                hQ.append(t_)
            hh = []
            for tc_ in range(4):
                t_ = hpool.tile([128, E], F32, tag="hh", name=f"hh{tc_}")
                nc.sync.dma_start(out=t_[:], in_=h0h[tc_ * 128:(tc_ + 1) * 128, :])
                hh.append(t_)

            for l in range(n_layers):
                # ---- per-layer weight/bias loads (few big DMAs) ----
                QW = wpool.tile([128, 6176], DT, tag="qkvw", name=f"QW{l}")
                nc.sync.dma_start(out=QW[:], in_=qkvw[l])
                WO = wpool.tile([128, 2048], DT, tag="wo1", name=f"WO{l}")
                nc.sync.dma_start(out=WO[:], in_=wo1[l])
                FW = wpool.tile([128, 16384], DT, tag="ffnw", name=f"FW{l}")
                nc.sync.dma_start(out=FW[:], in_=ffnw[l])
                BT = bpool.tile([128, 104], F32, tag="b128", name=f"BT{l}")
                nc.sync.dma_start(out=BT[:], in_=b128[l])
                RB = bpool.tile([1, 1544], DT, tag="brow", name=f"RB{l}")
                nc.sync.dma_start(out=RB[:], in_=brow[l])
                if affine:
                    LNW = bpool.tile([128, 4, E], F32, tag="lnw", bufs=1,
                                     name=f"LNW{l}")
                    nc.sync.dma_start(out=LNW[:], in_=lnw[l:l + 1].broadcast_to(
                        (128, 4, E)))
                else:
                    LNW = None

                def wq_s(p, c):
                    return QW[:, p * 512 + c * 128:p * 512 + (c + 1) * 128]

                def wk_s(p, c):
                    return QW[:, 2048 + p * 512 + c * 128:
                              2048 + p * 512 + (c + 1) * 128]

                def wv_s(p, c):
                    o = 4096 + p * 520 + c * 130
                    return QW[:, o:o + 130]

                # ---- q/k projections (head pairs) ----
                qT, kT = [], []
                for p in range(4):
                    pq = pp.tile([128, Tq], F32, tag="pacc", name=f"pq{l}_{p}")
                    for ec in range(4):
                        nc.tensor.matmul(pq[:], wq_s(p, ec), hQ[ec][:],
                                         start=ec == 0, stop=ec == 3)
                    q_t = apool.tile([128, Tq], DT, tag="qT", bufs=4,
                                     name=f"qTt{l}_{p}")
                    if biases:
                        nc.scalar.activation(out=q_t[:], in_=pq[:],
                                             func=AF.Identity, bias=BT[:, p:p + 1])
                    else:
                        nc.scalar.activation(out=q_t[:], in_=pq[:], func=AF.Copy)
                    qT.append(q_t)

                    k_t = apool.tile([128, T], DT, tag="kT", bufs=4,
                                     name=f"kTt{l}_{p}")
                    for half in range(2):
                        pk = pp.tile([128, 512], F32, tag="pacc",
                                     name=f"pk{l}_{p}_{half}")
                        for ec in range(4):
                            nc.tensor.matmul(pk[:], wk_s(p, ec),
                                             hT[ec][:, half * 512:(half + 1) * 512],
                                             start=ec == 0, stop=ec == 3)
                        if biases:
                            nc.scalar.activation(
                                out=k_t[:, half * 512:(half + 1) * 512],
                                in_=pk[:], func=AF.Identity,
                                bias=BT[:, 4 + p:5 + p])
                        else:
                            nc.scalar.activation(
                                out=k_t[:, half * 512:(half + 1) * 512],
                                in_=pk[:], func=AF.Copy)
                    kT.append(k_t)

                # ---- attention + SAE per head ----
                opair = []
                vpair = [None] * 4
                for h in range(H):
                    p, off = h // 2, (h % 2) * 64
                    if h % 2 == 0:
                        # v projection for the pair (ones-augmented cols 64/129;
                        # bias + ones supplied by a K=1 ones-row matmul)
                        vp_t = apool.tile([128, 8, 130], DT, tag="vp", bufs=3,
                                          name=f"vp{l}_{p}")
                        for g in range(4):
                            pv = pp.tile([128, 2, 130], F32, tag="pacc",
                                         name=f"pv{l}_{p}_{g}")
                            for j in range(2):
                                sc = 2 * g + j
                                for ec in range(4):
                                    nc.tensor.matmul(
                                        pv[:, j, :],
                                        hT[ec][:, sc * 128:(sc + 1) * 128],
                                        wv_s(p, ec), start=ec == 0, stop=False)
                                nc.tensor.matmul(pv[:, j, :], ones_col[:],
                                                 RB[:, p * 130:(p + 1) * 130],
                                                 start=False, stop=True)
                            nc.scalar.activation(out=vp_t[:, 2 * g:2 * g + 2, :],
                                                 in_=pv[:], func=AF.Copy)
                        vpair[p] = vp_t
                    vp_t = vpair[p]

                    SW = wpool.tile([128, 4096], DT, tag="saew", bufs=2,
                                    name=f"SW{l}_{h}")
                    nc.sync.dma_start(out=SW[:], in_=saew[l, h])

                    # scores rawT[s, t] (1/sqrt(hd) folded into wq)
                    raw_t = apool.tile([128, 8, 512], DT, tag="raw", bufs=1,
                                       name=f"raw{l}_{h}")
                    for sc in range(8):
                        pr = pp.tile([128, 512], F32, tag="pacc",
                                     name=f"praw{l}_{h}_{sc}")
                        nc.tensor.matmul(pr[:],
                                         kT[p][off:off + 64, sc * 128:(sc + 1) * 128],
                                         qT[p][off:off + 64, :],
                                         start=True, stop=True)
                        nc.vector.tensor_copy(out=raw_t[:, sc, :], in_=pr[:])

                    # SAE encode: latT[u, t] = relu(we^T rawT + be) on DVE
                    plat = pp.tile([128, 2, 512], F32, tag="plat", bufs=1,
                                   name=f"plat{l}_{h}")
                    for uc in range(2):
                        for sc in range(8):
                            nc.tensor.matmul(
                                plat[:, uc, :],
                                SW[:, sc * 256 + uc * 128:sc * 256 + (uc + 1) * 128],
                                raw_t[:, sc, :], start=sc == 0, stop=sc == 7)
                    lat_t = apool.tile([128, 2, 512], DT, tag="lat", bufs=2,
                                       name=f"lat{l}_{h}")
                    for uc in range(2):
                        if biases:
                            nc.vector.tensor_scalar(
                                out=lat_t[:, uc, :], in0=plat[:, uc, :],
                                scalar1=BT[:, 8 + h * 2 + uc:9 + h * 2 + uc],
                                scalar2=0.0, op0=ALU.add, op1=ALU.max)
                        else:
                            nc.vector.tensor_scalar(
                                out=lat_t[:, uc, :], in0=plat[:, uc, :],
                                scalar1=0.0, scalar2=None, op0=ALU.max)

                    # decode -> sigmoid (via tanh) -> exp -> attn x V
                    pot = pp.tile([65, 512], F32, tag="pot", bufs=2,
                                  name=f"pot{l}_{h}")
                    for sc in range(8):
                        prc = pp.tile([128, 512], F32, tag="pacc",
                                      name=f"prc{l}_{h}_{sc}")
                        for uc in range(2):
                            nc.tensor.matmul(
                                prc[:],
                                SW[:, 2048 + uc * 1024 + sc * 128:
                                   2048 + uc * 1024 + (sc + 1) * 128],
                                lat_t[:, uc, :], start=uc == 0, stop=uc == 1)
                        # sigmoid(z+bd) = 0.5 + 0.5*tanh((z+bd)/2); table set
                        # keeps tanh+exp resident together
                        th_t = apool.tile([128, 512], DT, tag="tht", bufs=2,
                                          name=f"th{l}_{h}_{sc}")
                        if biases:
                            nc.scalar.activation(out=th_t[:], in_=prc[:],
                                                 func=AF.Tanh, scale=0.5,
                                                 bias=BT[:, 24 + h * 8 + sc:
                                                         25 + h * 8 + sc])
                        else:
                            nc.scalar.activation(out=th_t[:], in_=prc[:],
                                                 func=AF.Tanh, scale=0.5)
                        exp_t = apool.tile([128, 512], DT, tag="expt", bufs=2,
                                           name=f"exp{l}_{h}_{sc}")
                        nc.scalar.activation(out=exp_t[:], in_=th_t[:], func=AF.Exp,
                                             scale=0.5, bias=half_t[:])
                        o2 = (h % 2) * 65
                        nc.tensor.matmul(pot[:], vp_t[:, sc, o2:o2 + 65], exp_t[:],
                                         start=sc == 0, stop=sc == 7)

                    rr = spool.tile([1, 512], F32, tag="rr", name=f"rr{l}_{h}")
                    nc.vector.reciprocal(out=rr[:], in_=pot[64:65, :])
                    rb = spool.tile([64, 512], F32, tag="rb", name=f"rb{l}_{h}")
                    nc.gpsimd.partition_broadcast(rb[:], rr[:])
                    if off == 0:
                        op_t = apool.tile([128, Tq], DT, tag="op", bufs=5,
                                          name=f"op{l}_{p}")
                        opair.append(op_t)
                    nc.vector.tensor_mul(out=opair[p][off:off + 64, :],
                                         in0=pot[0:64, :], in1=rb[:])

                # ---- output projection + LN1 ----
                h1f = []
                h1T = [hpool.tile([128, Tq], DT, tag="h1T", bufs=5,
                                  name=f"h1T{l}_{ec}") for ec in range(4)]
                for tc_ in range(4):
                    pm = pp.tile([128, E], F32, tag="pacc", name=f"pm{l}_{tc_}")
                    for p in range(4):
                        nc.tensor.matmul(pm[:],
                                         opair[p][:, tc_ * 128:(tc_ + 1) * 128],
                                         WO[:, p * 512:(p + 1) * 512],
                                         start=p == 0,
                                         stop=(not biases) and p == 3)
                    if biases:
                        nc.tensor.matmul(pm[:], ones_col[:], RB[:, 520:1032],
                                         start=False, stop=True)
                    x1 = spool.tile([128, E], F32, tag="x1", name=f"x1_{l}_{tc_}")
                    nc.vector.tensor_add(out=x1[:], in0=pm[:], in1=hh[tc_][:])
                    y1 = hpool.tile([128, E], F32, tag="h1f", name=f"h1f{l}_{tc_}")
                    ln_apply(x1[:], y1, LNW[:, 0, :] if affine else None,
                             LNW[:, 1, :] if affine else None)
                    h1f.append(y1)
                if True:
                    transpose_rows(h1f, h1T, f"a{l}")

                # ---- FFN ----
                aT = []
                for fc in range(16):
                    pa = pp.tile([128, Tq], F32, tag="pacc", name=f"pa{l}_{fc}")
                    for ec in range(4):
                        nc.tensor.matmul(pa[:],
                                         FW[:, ec * 2048 + fc * 128:
                                            ec * 2048 + (fc + 1) * 128],
                                         h1T[ec][:], start=ec == 0, stop=ec == 3)
                    a_t = apool.tile([128, Tq], DT, tag="aT", bufs=17,
                                     name=f"aT{l}_{fc}")
                    if biases:
                        nc.scalar.activation(out=a_t[:], in_=pa[:], func=AF.Relu,
                                             bias=BT[:, 88 + fc:89 + fc])
                    else:
                        nc.scalar.activation(out=a_t[:], in_=pa[:], func=AF.Relu)
                    aT.append(a_t)

                hh_new = []
                for tc_ in range(4):
                    pb = pp.tile([128, E], F32, tag="pacc", name=f"pb{l}_{tc_}")
                    for fc in range(16):
                        nc.tensor.matmul(pb[:], aT[fc][:, tc_ * 128:(tc_ + 1) * 128],
                                         FW[:, 8192 + fc * 512:8192 + (fc + 1) * 512],
                                         start=fc == 0,
                                         stop=(not biases) and fc == 15)
                    if biases:
                        nc.tensor.matmul(pb[:], ones_col[:], RB[:, 1032:1544],
                                         start=False, stop=True)
                    x2 = spool.tile([128, E], F32, tag="x2", name=f"x2_{l}_{tc_}")
                    nc.scalar.activation(out=x2[:], in_=pb[:], func=AF.Relu)
                    nc.vector.tensor_add(out=x2[:], in0=x2[:], in1=h1f[tc_][:])
                    y2 = hpool.tile([128, E], F32, tag="hh", name=f"hh{l}_{tc_}")
                    ln_apply(x2[:], y2, LNW[:, 2, :] if affine else None,
                             LNW[:, 3, :] if affine else None)
                    hh_new.append(y2)
                    if l == n_layers - 1:
                        nc.sync.dma_start(
                            out=hout[tc_ * 128:(tc_ + 1) * 128, :], in_=y2[:])
                    else:
                        y2d = hpool.tile([128, E], DT, tag="h2d", bufs=2,
                                         name=f"h2d{l}_{tc_}")
                        nc.vector.tensor_copy(out=y2d[:], in_=y2[:])
                        nc.sync.dma_start(
                            out=agin[l][tc_ * 128:(tc_ + 1) * 128, :], in_=y2d[:])
                hh = hh_new

                if l < n_layers - 1:
                    hQ = [hpool.tile([128, Tq], DT, tag="hQ", name=f"hQn{l}_{ec}")
                          for ec in range(4)]
                    transpose_rows(hh_new, hQ, f"q{l}")
                    nc.gpsimd.collective_compute(
                        "AllGather", ALU.bypass, replica_groups=groups,
                        ins=[agin[l][:]], outs=[agout[l][:]])
                    hT = []
                    for ec in range(4):
                        t_ = hpool.tile([128, T], DT, tag="hT", name=f"hTn{l}_{ec}")
                        if DT == mybir.dt.bfloat16:
                            nc.sync.dma_start_transpose(
                                t_[:], agout[l][:, ec * 128:(ec + 1) * 128])
                        else:
                            nc.sync.dma_start(
                                out=t_[:],
                                in_=agout[l][:, ec * 128:(ec + 1) * 128]
                                .rearrange("a b -> b a"))
                        hT.append(t_)

    nc.finalize()
    return nc


def _get_nc(n_layers=L, use_bf16=True, affine=False, biases=False):
    key = (n_layers, use_bf16, affine, biases)
    if key not in _BUILD_CACHE:
        _BUILD_CACHE[key] = _build(n_layers, use_bf16, affine, biases)
    return _BUILD_CACHE[key]


def _prep_host(tok_emb, pos_emb, wq, bq, wk, bk, wv, bv, we, be, wd, bd,
               wo, bo, ln1_g, ln1_b, w1, b1, w2, b2, ln2_g, ln2_b, x, mask,
               n_layers=L, use_bf16=True):
    """Shard + retile + retype inputs for the 8 cores."""
    npdt = ml_dtypes.bfloat16 if use_bf16 else np.float32
    f32 = np.float32

    tok_emb = np.asarray(tok_emb, f32)
    pos_emb = np.asarray(pos_emb, f32)
    x = np.asarray(x)
    h0 = tok_emb[x] + pos_emb[None, :, :]          # [B,T,E] f32

    nl = n_layers
    wq_s = np.asarray(wq, f32)[:nl] * SCALE        # [l,H,E,HD]
    wk_s = np.asarray(wk, f32)[:nl]
    # paired q/k: [l,4,E,128] -> [l,4,4(c),128(r),128(m)] -> [l,r,p,c,m]
    def pair_tile(w):
        w2 = w.reshape(nl, 4, 2, E, HD).transpose(0, 1, 3, 2, 4).reshape(
            nl, 4, E, 128)
        w2 = w2.reshape(nl, 4, 4, 128, 128).transpose(0, 3, 1, 2, 4)
        return w2.reshape(nl, 128, 2048)

    qkvw = np.zeros((nl, 128, 6176), f32)
    qkvw[:, :, 0:2048] = pair_tile(wq_s)
    qkvw[:, :, 2048:4096] = pair_tile(wk_s)
    wva = np.concatenate([np.asarray(wv, f32)[:nl],
                          np.zeros((nl, H, E, 1), f32)], axis=-1)  # [l,H,E,65]
    # pair-stack: [l,4(pair),E,130] -> [l,4,4(c),128(r),130] -> [l,r,p,c,130]
    wvp = wva.reshape(nl, 4, 2, E, 65).transpose(0, 1, 3, 2, 4).reshape(
        nl, 4, E, 130)
    wvp = wvp.reshape(nl, 4, 4, 128, 130).transpose(0, 3, 1, 2, 4)
    qkvw[:, :, 4096:6176] = wvp.reshape(nl, 128, 4 * 4 * 130)
    qkvw = np.ascontiguousarray(qkvw).astype(npdt)

    wed = np.asarray(we, f32)[:nl]                 # [l,H,1024,256]
    wdd = np.asarray(wd, f32)[:nl]                 # [l,H,256,1024]
    saew = np.zeros((nl, H, 128, 4096), f32)
    saew[:, :, :, 0:2048] = wed.reshape(nl, H, 8, 128, 256).transpose(
        0, 1, 3, 2, 4).reshape(nl, H, 128, 2048)
    saew[:, :, :, 2048:4096] = wdd.reshape(nl, H, 2, 128, 1024).transpose(
        0, 1, 3, 2, 4).reshape(nl, H, 128, 2048)
    saew = np.ascontiguousarray(saew).astype(npdt)

    wo_ = np.asarray(wo, f32)[:nl]                 # [l,512,512]
    wo1 = np.ascontiguousarray(
        wo_.reshape(nl, 4, 128, 512).transpose(0, 2, 1, 3).reshape(
            nl, 128, 2048)).astype(npdt)

    w1_ = np.asarray(w1, f32)[:nl]                 # [l,E,F]
    w2_ = np.asarray(w2, f32)[:nl]                 # [l,F,E]
    ffnw = np.zeros((nl, 128, 16384), f32)
    ffnw[:, :, 0:8192] = w1_.reshape(nl, 4, 128, F).transpose(
        0, 2, 1, 3).reshape(nl, 128, 8192)
    ffnw[:, :, 8192:16384] = w2_.reshape(nl, 16, 128, E).transpose(
        0, 2, 1, 3).reshape(nl, 128, 8192)
    ffnw = np.ascontiguousarray(ffnw).astype(npdt)

    b128 = np.zeros((nl, 128, 104), f32)
    b128[:, :, 0:4] = (np.asarray(bq, f32)[:nl] * SCALE).reshape(
        nl, 4, 128).transpose(0, 2, 1)
    b128[:, :, 4:8] = np.asarray(bk, f32)[:nl].reshape(
        nl, 4, 128).transpose(0, 2, 1)
    b128[:, :, 8:24] = np.asarray(be, f32)[:nl].reshape(
        nl, H * 2, 128).transpose(0, 2, 1)
    b128[:, :, 24:88] = 0.5 * np.asarray(bd, f32)[:nl].reshape(
        nl, H * 8, 128).transpose(0, 2, 1)
    b128[:, :, 88:104] = np.asarray(b1, f32)[:nl].reshape(
        nl, 16, 128).transpose(0, 2, 1)

    brow = np.zeros((nl, 1, 1544), f32)
    bva = np.concatenate([np.asarray(bv, f32)[:nl],
                          np.ones((nl, H, 1), f32)], axis=-1)      # [l,H,65]
    brow[:, 0, 0:520] = bva.reshape(nl, 520)
    brow[:, 0, 520:1032] = np.asarray(bo, f32)[:nl]
    brow[:, 0, 1032:1544] = np.asarray(b2, f32)[:nl]
    brow = brow.astype(npdt)

    lnw = np.ascontiguousarray(np.stack(
        [np.asarray(ln1_g, f32)[:nl], np.asarray(ln1_b, f32)[:nl],
         np.asarray(ln2_g, f32)[:nl], np.asarray(ln2_b, f32)[:nl]],
        axis=1))                                                   # [l,4,E]

    shared = dict(qkvw=qkvw, saew=saew, wo1=wo1, ffnw=ffnw,
                  b128=b128, brow=brow, lnw=lnw)

    in_maps = []
    for c in range(NCORES):
        b_, half = c // 2, c % 2
        hT0 = np.ascontiguousarray(h0[b_].T).astype(npdt)          # [E,T]
        qhT0 = np.ascontiguousarray(
            h0[b_, half * Tq:(half + 1) * Tq, :].T).astype(npdt)    # [E,Tq]
        h0h = np.ascontiguousarray(
            h0[b_, half * Tq:(half + 1) * Tq, :]).astype(f32)       # [Tq,E]
        in_maps.append(dict(hT0=hT0, qhT0=qhT0, h0h=h0h, **shared))
    return in_maps


def kernel(**inputs):
    n_layers = inputs.pop("_n_layers", L)
    use_bf16 = inputs.pop("_use_bf16", True)
    affine = not (np.all(np.asarray(inputs["ln1_g"]) == 1.0)
                  and np.all(np.asarray(inputs["ln1_b"]) == 0.0)
                  and np.all(np.asarray(inputs["ln2_g"]) == 1.0)
                  and np.all(np.asarray(inputs["ln2_b"]) == 0.0))
    biases = any(np.any(np.asarray(inputs[k]) != 0.0)
                 for k in ("bq", "bk", "bv", "be", "bd", "bo", "b1", "b2"))
    nc = _get_nc(n_layers, use_bf16, affine, biases)
    in_maps = _prep_host(**inputs, n_layers=n_layers, use_bf16=use_bf16)
    res = run_bass_kernel_spmd(nc, in_maps, list(range(NCORES)))
    h = np.zeros((B, T, E), np.float32)
    for c in range(NCORES):
        b_, half = c // 2, c % 2
        h[b_, half * Tq:(half + 1) * Tq, :] = res.results[c]["hout"]
    # softmax rows each sum to exactly 1 -> loss = LAM * (#rows) per layer
    loss = np.float32(LAM * n_layers * B * H * T)
    return h, loss
